# revision 2
# baseline (speedup 1.0000x reference)
"""Elman RNN (DummyRNN) Trainium2 Bass kernel.

Math: h_t = tanh(x_t @ Ww.T + h_{t-1} @ Uw.T + (Wb + Ub + b)), t = 0..T-1
Output: concat over t of h_t  -> [T*B, D_OUT]

Strategy (data-parallel over batch, 8 cores, B_local = 8):
  Phase A: Z = X_local @ Ww.T + bias, batched over all T*B_local rows
           (dense full-array matmuls), stored to internal DRAM in a
           compact per-strip layout zbuf[t, g, b, j] = z[t, b, 512g+j].
  Phase B: sequential recurrence. Output features split into 4 column
           strips of the PE array (tile_position=(0,32g)); per round
           the 4 strips' matmuls are issued back-to-back (k-outer,
           g-inner) so they stream concurrently through disjoint
           32-column groups — per-step span ~16 x 512 cycles instead
           of 64 x 512.
           z_t is injected first via an identity-stationary matmul
           (start=True), U-chunk matmuls accumulate after.
           tanh runs as 4 column-quarter ACT ops; after quarter q the
           4 chunks {q, q+4, q+8, q+12} are PE-transposed concurrently
           (distinct row strips) and copied to SBUF by one DVE op,
           so the next step's stationary tiles appear incrementally.
           Next step processes chunks in quarter order, shrinking the
           inter-step bubble to ~tanh(q0)+transpose(q0).
  All matmul operands bf16 (fp32 PSUM accumulate); the recurrence is
  contractive so per-step rounding does not amplify (~4e-3 rel).
  Host pre-transposes X/Ww/Uw so no on-chip input transposes needed.
"""

import sys

for _p in ("/opt/trn_rl_repo",):
    if _p not in sys.path:
        sys.path.insert(0, _p)

import numpy as np
import ml_dtypes

import concourse.bass as bass
import concourse.bacc as bacc
import concourse.tile as tile
from concourse import mybir
from concourse.bass_utils import run_bass_kernel_spmd

BF16 = ml_dtypes.bfloat16

T, B, DIN, DOUT = 512, 64, 1024, 2048
NCORES = 8
BL = B // NCORES          # batch rows per core (8)
P = 128                   # partitions
NG = 4                    # column strips
GW = DOUT // NG           # features per strip (512)
KCH = DOUT // P           # U contraction chunks (16)
KCH_W = DIN // P          # Ww contraction chunks (8)
TPB = P // NG             # partitions per strip slot (32)


def _build_nc(t_steps: int) -> bass.Bass:
    nc = bacc.Bacc()
    dt = mybir.dt
    TANH = mybir.ActivationFunctionType.Tanh

    rows = t_steps * BL
    n_mtiles = rows // P
    assert rows % P == 0
    t_per_mtile = P // BL     # 16 timesteps per phase-A m-tile

    xT = nc.dram_tensor("xt", [DIN, rows], dt.bfloat16, kind="ExternalInput")
    wwT = nc.dram_tensor("wwt", [DIN, DOUT], dt.bfloat16, kind="ExternalInput")
    uT = nc.dram_tensor("ut", [DOUT, DOUT], dt.bfloat16, kind="ExternalInput")
    biasr = nc.dram_tensor("biasr", [1, DOUT], dt.bfloat16, kind="ExternalInput")
    # identity blocks at partition offsets 32g (z-add + transposes)
    identb = nc.dram_tensor("identb", [P, BL], dt.bfloat16, kind="ExternalInput")
    ones = nc.dram_tensor("ones", [1, P], dt.bfloat16, kind="ExternalInput")
    ys = nc.dram_tensor("ys", [rows, DOUT], dt.bfloat16, kind="ExternalOutput")

    with tile.TileContext(nc) as tc:
        with (
            tc.tile_pool(name="const", bufs=1) as const,
            tc.tile_pool(name="dram", bufs=1, space="DRAM") as dram,
            tc.tile_pool(name="aweights", bufs=1) as aweights,
            tc.tile_pool(name="xt_pool", bufs=2) as xt_pool,
            tc.tile_pool(name="zout", bufs=3) as zout,
            tc.tile_pool(name="psumA", bufs=2, space="PSUM") as psumA,
            tc.tile_pool(name="u_res", bufs=1) as u_res,
            tc.tile_pool(name="hpool", bufs=2) as hpool,
            tc.tile_pool(name="hTp", bufs=2) as hTp,
            tc.tile_pool(name="ztp", bufs=4) as ztp,
            tc.tile_pool(name="dvew", bufs=2) as dvew,
            tc.tile_pool(name="psB", bufs=2, space="PSUM") as psB,
            tc.tile_pool(name="psT", bufs=1, space="PSUM") as psT,
        ):
            identb_sb = const.tile([P, BL], dt.bfloat16)
            nc.sync.dma_start(out=identb_sb, in_=identb[:, :])
            # zbuf[t, g, b, j] = z[t, b, 512 g + j]
            zbuf = dram.tile([t_steps, NG, BL, GW], dt.bfloat16)

            wwT_sb = aweights.tile([P, KCH_W, DOUT], dt.bfloat16)
            for k in range(KCH_W):
                nc.sync.dma_start(
                    out=wwT_sb[:, k, :], in_=wwT[k * P:(k + 1) * P, :]
                )
            bias_sb = aweights.tile([1, DOUT], dt.bfloat16)
            nc.sync.dma_start(out=bias_sb, in_=biasr[:, :])
            ones_sb = aweights.tile([1, P], dt.bfloat16)
            nc.sync.dma_start(out=ones_sb, in_=ones[:, :])

            # ---- Phase A units (folded into the recurrence tail) ----
            # unit (m, g) computes z[:, 512g:512(g+1)] for timesteps
            # [16m, 16m+16) and stores to zbuf.  m-tile 0 runs up front;
            # m-tile m (m>=1) runs in the PE bubbles of steps
            # 16(m-1)+0 .. 16(m-1)+3 — long before step 16m needs it.
            def load_xt(m: int):
                tiles = []
                for k in range(KCH_W):
                    xt_t = xt_pool.tile(
                        [P, P], dt.bfloat16, tag=f"xt{k}", name=f"xt{k}"
                    )
                    nc.sync.dma_start(
                        out=xt_t,
                        in_=xT[k * P:(k + 1) * P, m * P:(m + 1) * P],
                    )
                    tiles.append(xt_t)
                return tiles

            def emit_unit(m: int, g: int, xt_tiles, scalar_copy: bool):
                sl = slice(g * GW, (g + 1) * GW)
                psa = psumA.tile([P, GW], dt.float32, tag="psA", name="psA")
                for k in range(KCH_W):
                    nc.tensor.matmul(
                        psa,
                        xt_tiles[k],
                        wwT_sb[:, k, sl],
                        start=(k == 0),
                        stop=False,
                    )
                nc.tensor.matmul(
                    psa, ones_sb, bias_sb[:, sl], start=False, stop=True,
                )
                zt_o = zout.tile([P, GW], dt.bfloat16, tag="zo", name="zo")
                if scalar_copy:
                    # keep DVE free for the latency-critical hT copy
                    nc.scalar.copy(zt_o, psa)
                else:
                    nc.vector.tensor_copy(zt_o, psa)
                dst = bass.AP(
                    tensor=zbuf.tensor,
                    offset=zbuf.offset
                    + (m * t_per_mtile) * (NG * BL * GW)
                    + g * (BL * GW),
                    ap=[[NG * BL * GW, t_per_mtile], [GW, BL], [1, GW]],
                )
                nc.sync.dma_start(out=dst, in_=zt_o)

            xt_cur = load_xt(0)
            for g in range(NG):
                emit_unit(0, g, xt_cur, scalar_copy=False)

            uT_sb = u_res.tile([P, KCH, DOUT], dt.bfloat16)
            for k in range(KCH):
                nc.sync.dma_start(
                    out=uT_sb[:, k, :], in_=uT[k * P:(k + 1) * P, :]
                )

            n_zgrp = (t_steps + 3) // 4
            zt_tiles: dict[int, object] = {}

            def load_zgrp(j: int) -> None:
                nts = min(4, t_steps - 4 * j)
                zt = ztp.tile([P, 4, GW], dt.bfloat16, tag="zt")
                for g in range(NG):
                    src = bass.AP(
                        tensor=zbuf.tensor,
                        offset=zbuf.offset + (4 * j * NG + g) * (BL * GW),
                        ap=[[GW, BL], [NG * BL * GW, nts], [1, GW]],
                    )
                    nc.sync.dma_start(
                        out=zt[TPB * g:TPB * g + BL, :nts, :], in_=src
                    )
                zt_tiles[j] = zt

            for j in range(min(3, n_zgrp)):
                load_zgrp(j)

            hT_prev = None
            ps_cur = None
            for t in range(t_steps):
                if t % 4 == 0 and (t // 4 + 3) < n_zgrp:
                    load_zgrp(t // 4 + 3)
                zt = zt_tiles[t // 4]

                if ps_cur is None:
                    ps_cur = psB.tile(
                        [P, GW], dt.float32, tag="psB", name="psB"
                    )
                ps = ps_cur
                if t < 2:
                    # Bootstrap: identity-matmul z injection with
                    # start=True sets every element's has_written bit
                    # in this bank (per-tile-partition-scoped clear).
                    for g in range(NG):
                        nc.tensor.matmul(
                            ps[TPB * g:TPB * g + BL, :],
                            identb_sb[TPB * g:TPB * g + BL, :],
                            zt[TPB * g:TPB * g + BL, t % 4, :],
                            start=True,
                            stop=(t == 0),
                            tile_position=(TPB * g, TPB * g),
                        )
                # For t >= 2, z was prewritten into ps by ScalarE
                # during step t-1 (below); the chains run start=False
                # and accumulate onto it via the has_written bits left
                # set by step t-2's chains in this same bank.
                if t > 0:
                    # round r: chunk m = (r%4)*4 + r//4 (quarter-major)
                    for r in range(KCH):
                        q, i = r // 4, r % 4
                        mchunk = q + 4 * i
                        for g in range(NG):
                            nc.tensor.matmul(
                                ps[TPB * g:TPB * g + BL, :],
                                hT_prev[:, i, q, :],
                                uT_sb[:, mchunk, g * GW:(g + 1) * GW],
                                start=False,
                                stop=(r == KCH - 1),
                                skip_group_check=True,
                                tile_position=(0, TPB * g),
                            )

                # Folded phase-A unit: fills this step's PE tail bubble.
                if t % 16 == 0 and t // 16 + 1 < n_mtiles:
                    xt_cur = load_xt(t // 16 + 1)
                if t % 16 < NG and t // 16 + 1 < n_mtiles:
                    emit_unit(t // 16 + 1, t % 16, xt_cur, scalar_copy=True)

                # Prewrite z for step t+1 while ScalarE is idle during
                # this step's span (queued before this step's tanh).
                if t + 1 < t_steps:
                    ps_nxt = psB.tile(
                        [P, GW], dt.float32, tag="psB", name="psB"
                    )
                    if t + 1 >= 2:
                        zt_n = zt_tiles[(t + 1) // 4]
                        nc.scalar.copy(ps_nxt, zt_n[:, (t + 1) % 4, :])
                    ps_cur = ps_nxt
                else:
                    ps_cur = None

                h = hpool.tile([P, GW], dt.bfloat16, tag="h")
                nc.scalar.activation(h, ps, TANH)
                if t < t_steps - 1:
                    # hT_next[p, i, q, b] = h[32i+b, 128q+p]
                    #                     = stationary chunk m = q+4i.
                    hT_next = hTp.tile(
                        [P, NG, NG, BL], dt.bfloat16, tag="hT", name="hT"
                    )
                    # One batched transpose block per step: 16 PE
                    # transposes into a single 4-bank PSUM tile (bank
                    # per row strip i — concurrent row tiles must hit
                    # different banks), then ONE strided DVE copy out.
                    pst = psT.tile(
                        [P, NG, P, BL], dt.bfloat16, tag="pst", name="pst"
                    )
                    # Warm the DVE while the transposes run so the hT
                    # copy dispatches promptly (cold DVE wake ~0.5us).
                    wk = dvew.tile([1, BL], dt.bfloat16, tag="wk", name="wk")
                    nc.vector.tensor_copy(wk, h[0:1, 0:BL])
                    for q in range(NG):
                        for i in range(NG):
                            nc.tensor.transpose(
                                pst[:, i, q, :],
                                h[TPB * i:TPB * i + BL, q * P:(q + 1) * P],
                                identb_sb[TPB * i:TPB * i + BL, :],
                                tile_position=(TPB * i, 0),
                            )
                    nc.vector.tensor_copy(hT_next, pst[:, :, 0:NG, :])
                else:
                    hT_next = None
                for g in range(NG):
                    nc.sync.dma_start(
                        out=ys[t * BL:(t + 1) * BL, g * GW:(g + 1) * GW],
                        in_=h[TPB * g:TPB * g + BL, :],
                    )
                hT_prev = hT_next

    nc.compile()
    return nc


_NC_CACHE: dict[int, bass.Bass] = {}
LAST_EXEC_NS = None
LAST_PROFILE = None


def _prep_inputs(x, Ww, Uw, bias, t_steps):
    wwT = np.ascontiguousarray(Ww.T).astype(BF16)          # [DIN, DOUT]
    uT = np.ascontiguousarray(Uw.T).astype(BF16)           # [DOUT, DOUT]
    biasr = bias.reshape(1, DOUT).astype(BF16)
    identb = np.zeros((P, BL), dtype=BF16)
    for j in range(NG):
        for c in range(BL):
            identb[j * TPB + c, c] = 1
    ones = np.ones((1, P), dtype=BF16)

    in_maps = []
    for c in range(NCORES):
        xl = x[:, c * BL:(c + 1) * BL, :].reshape(t_steps * BL, DIN)
        xTl = np.ascontiguousarray(xl.T).astype(BF16)      # [DIN, rows]
        in_maps.append(
            dict(xt=xTl, wwt=wwT, ut=uT, biasr=biasr, identb=identb,
                 ones=ones)
        )
    return in_maps


def kernel(input_data, Ww, Wb, Uw, Ub, b, concatenate=1, _t_steps=None,
           _trace=False):
    x = np.asarray(input_data, dtype=np.float32)
    if _t_steps is not None:
        x = x[:_t_steps]
    Ww = np.asarray(Ww, dtype=np.float32)
    Uw = np.asarray(Uw, dtype=np.float32)
    bias = (
        np.asarray(Wb, dtype=np.float32)
        + np.asarray(Ub, dtype=np.float32)
        + np.asarray(b, dtype=np.float32)
    )

    t_steps = x.shape[0]
    if t_steps not in _NC_CACHE:
        _NC_CACHE[t_steps] = _build_nc(t_steps)
    nc = _NC_CACHE[t_steps]

    in_maps = _prep_inputs(x, Ww, Uw, bias, t_steps)

    global LAST_EXEC_NS, LAST_PROFILE
    res = run_bass_kernel_spmd(
        nc, in_maps, core_ids=list(range(NCORES)), trace=_trace
    )
    LAST_EXEC_NS = res.exec_time_ns
    LAST_PROFILE = res
    ys_full = np.concatenate(
        [
            np.asarray(res.results[c]["ys"], dtype=np.float32).reshape(
                t_steps, BL, DOUT
            )
            for c in range(NCORES)
        ],
        axis=1,
    )  # [T, B, DOUT]
    if concatenate:
        return ys_full.reshape(-1, DOUT)
    return ys_full
